# revision 28
# baseline (speedup 1.0000x reference)
"""NonLocalBlock (nn_NonLocalBlock_80221399155245) — Trainium2 Bass kernel.

Sharding: data-parallel over batch B=8, one batch item per NeuronCore.
Per-core pipeline (xf = x[b] as [C=256, N=4096]):
  theta = Wq @ xf, phi = Wk @ xf        [I=128, N]  (bf16, I-major)
  gT    = (Wg @ xf).T                   (N-major 128-chunks, bf16)
  per 256-column n-block (software-pipelined, deferred normalize chain):
    logits^T[m, n] = phi_m.T @ theta_n  (PE -> PSUM)
    P^T = exp(logits^T / sqrt(I))       (ACT, PSUM->SBUF bf16)
    denom = colsum(P^T)                 (DVE bf16 pairwise tree + PE ones-MM)
    outT  = sum_m gT_m.T @ P^T_m        (PE, PSUM-accumulated)
    on    = outT * (1/denom)            (PE bc-MM + GPSIMD mult)
    y     = Wo @ on                     (PE) -> GPSIMD drain to bf16 ysb,
                                        DVE bn_stats
  The per-block chain cs->recip->bc->on->ypj->drain is deferred 1-2 blocks
  so the PE instruction stream never stalls (keeps HAM clock-gate warm).
  BatchNorm batch-stats are AllReduced across the 8 cores (sync-BN exact),
  SE channel attention from x on-core; residual fused in the final apply.
"""

import numpy as np
import ml_dtypes
import concourse.bass as bass
import concourse.tile as tile
from concourse import bacc, mybir
from concourse.bass_utils import run_bass_kernel_spmd

F32 = mybir.dt.float32
BF16 = mybir.dt.bfloat16
AF = mybir.ActivationFunctionType
ALU = mybir.AluOpType

C = 256     # channels
I = 128     # inter channels
R = 64      # SE reduction
P = 128     # SBUF partitions
B = 8       # batch == cores
H = W = 64
N = H * W   # 4096 pixels
NB = 256    # n-block columns
CG = 4      # logits chunks per exp-activation group


def _build(n_cores=B, nn=N, nb=NB, total_pixels=None):
    M = nn // P            # 32 m-chunks
    NBLK = nn // nb        # 16 n-blocks
    GRP = M // CG          # 8 exp groups per block
    NPART = 4              # denominator partial trees per block (2 exp grps ea)
    if total_pixels is None:
        total_pixels = n_cores * nn
    sm_scale = float(1.0 / np.sqrt(np.float32(I)))

    nc = bacc.Bacc("TRN2", target_bir_lowering=False, debug=False,
                   num_devices=n_cores)

    x_d = nc.declare_dram_parameter("x", [C, nn], F32, isOutput=False)
    wq_d = nc.declare_dram_parameter("wq_t", [C, I], BF16, isOutput=False)
    wk_d = nc.declare_dram_parameter("wk_t", [C, I], BF16, isOutput=False)
    wg_d = nc.declare_dram_parameter("wg_t", [C, I], BF16, isOutput=False)
    wo_d = nc.declare_dram_parameter("wo_t", [I, C], BF16, isOutput=False)
    fc1w_d = nc.declare_dram_parameter("fc1_wt", [C, R], F32, isOutput=False)
    fc1b_d = nc.declare_dram_parameter("fc1_b", [R], F32, isOutput=False)
    fc2w_d = nc.declare_dram_parameter("fc2_wt", [R, C], F32, isOutput=False)
    fc2bn_d = nc.declare_dram_parameter("fc2_bn", [C], F32, isOutput=False)
    gam_d = nc.declare_dram_parameter("bn_gamma", [C], F32, isOutput=False)
    bet_d = nc.declare_dram_parameter("bn_beta", [C], F32, isOutput=False)
    out_d = nc.declare_dram_parameter("out", [C, nn], BF16, isOutput=True)

    bn_in = nc.dram_tensor("bn_in", [P, 4], F32)
    bn_out = nc.dram_tensor("bn_out", [P, 4], F32,
                            addr_space="Shared" if n_cores > 4 else "Local")

    with tile.TileContext(nc) as tc:
        import contextlib
        with contextlib.ExitStack() as stack:
            sing = stack.enter_context(tc.tile_pool(name="sing", bufs=1))

            # persistent SBUF tiles
            xf32 = [sing.tile([P, nn], F32, tag=f"xf32_{cc}", name=f"xf32_{cc}")
                    for cc in range(2)]
            xbf = [sing.tile([P, nn], BF16, tag=f"xbf_{cc}", name=f"xbf_{cc}")
                   for cc in range(2)]
            theta = sing.tile([P, nn], BF16, tag="theta", name="theta")
            phi = sing.tile([P, nn], BF16, tag="phi", name="phi")
            gT = sing.tile([P, nn], BF16, tag="gT", name="gT")
            ysb = sing.tile([P, 2, nn], BF16, tag="ysb", name="ysb")
            bnst = [sing.tile([P, NBLK, 6], F32, tag=f"bnst_{cc}",
                              name=f"bnst_{cc}") for cc in range(2)]

            wq = sing.tile([P, 2, I], BF16, tag="wq", name="wq")
            wk = sing.tile([P, 2, I], BF16, tag="wk", name="wk")
            wg = sing.tile([P, 2, I], BF16, tag="wg", name="wg")
            wo = sing.tile([P, 2, P], BF16, tag="wo", name="wo")
            fc1w = sing.tile([P, 2, R], F32, tag="fc1w", name="fc1w")
            fc1b = sing.tile([R, 1], F32, tag="fc1b", name="fc1b")
            fc2w = sing.tile([R, 2, P], F32, tag="fc2w", name="fc2w")
            fc2bn = sing.tile([P, 2], F32, tag="fc2bn", name="fc2bn")
            gam = sing.tile([P, 2], F32, tag="gam", name="gam")
            bet = sing.tile([P, 2], F32, tag="bet", name="bet")
            ones_cb = sing.tile([P, 1], BF16, tag="ones_cb", name="ones_cb")
            ones_row = sing.tile([1, P], F32, tag="ones_row", name="ones_row")
            chw = sing.tile([P, 2], F32, tag="chw", name="chw")
            warm_sb = sing.tile([P, 512], BF16, tag="warm_sb", name="warm_sb")
            pooled_q = sing.tile([P, 2, nn // 512], F32, tag="pooled_q",
                                 name="pooled_q")
            pooled = sing.tile([P, 2], F32, tag="pooled", name="pooled")

            nc.vector.memset(ones_cb, 1.0)
            nc.vector.memset(ones_row, 1.0)
            nc.vector.memset(warm_sb, 0.0)

            # attention weights first (tiny), then x in 512-col chunks split
            # across both HWDGE queues (sync + scalar) so the cast/proj
            # pipeline starts ~3us in; epilogue-only weights last
            nc.sync.dma_start(out=wq, in_=wq_d.rearrange("(a p) i -> p a i", p=P))
            nc.scalar.dma_start(out=wk, in_=wk_d.rearrange("(a p) i -> p a i", p=P))
            nc.sync.dma_start(out=wg, in_=wg_d.rearrange("(a p) i -> p a i", p=P))
            nc.scalar.dma_start(out=wo, in_=wo_d.rearrange("i (a c) -> i a c", a=2))

            NT = nn // 512
            for t in range(NT):
                for cc in range(2):
                    eng = nc.sync if cc == 0 else nc.scalar
                    eng.dma_start(
                        out=xf32[cc][:, t * 512:(t + 1) * 512],
                        in_=x_d[cc * P:(cc + 1) * P, t * 512:(t + 1) * 512])

            nc.sync.dma_start(out=fc1w, in_=fc1w_d.rearrange("(a p) r -> p a r", p=P))
            nc.sync.dma_start(out=fc1b, in_=fc1b_d[:, None])
            nc.sync.dma_start(out=fc2w, in_=fc2w_d.rearrange("r (a c) -> r a c", a=2))
            nc.sync.dma_start(out=fc2bn, in_=fc2bn_d.rearrange("(a p) -> p a", p=P))
            nc.sync.dma_start(out=gam, in_=gam_d.rearrange("(a p) -> p a", p=P))
            nc.sync.dma_start(out=bet, in_=bet_d.rearrange("(a p) -> p a", p=P))

            # ---- prologue: warmup + QKV projections + SE ----
            with tc.tile_pool(name="proj_ps", bufs=4, space="PSUM") as pps, \
                 tc.tile_pool(name="warm_ps", bufs=1, space="PSUM") as wps, \
                 tc.tile_pool(name="se_ps", bufs=1, space="PSUM") as seps:
                # HAM warm-up: junk matmuls during the x DMA wait
                warm_ps = wps.tile([P, 512], F32, tag="warm", name="warm")
                for w in range(10):
                    nc.tensor.matmul(warm_ps[:], warm_sb[:, 0:P], warm_sb[:],
                                     start=(w == 0), stop=(w == 9))

                for t in range(NT):
                    lo = t * 512
                    # cast to bf16 on ACT with fused pooled partial sums
                    for cc in range(2):
                        nc.scalar.activation(
                            xbf[cc][:, lo:lo + 512], xf32[cc][:, lo:lo + 512],
                            AF.Copy,
                            accum_out=pooled_q[:, cc, t:t + 1])
                    # theta/phi projections for this tile
                    for (wt, dst) in ((wq, theta), (wk, phi)):
                        ps = pps.tile([P, 512], F32, tag="proj", name="proj")
                        for cc in range(2):
                            nc.tensor.matmul(ps[:], wt[:, cc, :],
                                             xbf[cc][:, lo:lo + 512],
                                             start=(cc == 0), stop=(cc == 1))
                        nc.vector.tensor_copy(dst[:, lo:lo + 512], ps[:])
                    # gT for this tile (x-chunk stationary, wg moving)
                    ps = pps.tile([P, 4, I], F32, tag="proj", name="proj")
                    for j in range(4):
                        mj = t * 4 + j
                        for cc in range(2):
                            nc.tensor.matmul(
                                ps[:, j, :], xbf[cc][:, mj * P:(mj + 1) * P],
                                wg[:, cc, :], start=(cc == 0), stop=(cc == 1))
                    nc.vector.tensor_copy(
                        gT[:, t * 4 * I:(t + 1) * 4 * I],
                        ps[:, :, :].rearrange("p a b -> p (a b)"))

                # SE: pooled -> fc1 relu -> fc2 sigmoid (as 1/(1+exp(-z)))
                for cc in range(2):
                    nc.vector.reduce_sum(pooled[:, cc:cc + 1],
                                         pooled_q[:, cc, :],
                                         axis=mybir.AxisListType.X)
                hps = seps.tile([R, 1], F32, tag="se", name="se_h")
                for cc in range(2):
                    nc.tensor.matmul(hps[:], fc1w[:, cc, :], pooled[:, cc:cc + 1],
                                     start=(cc == 0), stop=(cc == 1))
                hsb = sing.tile([R, 1], F32, tag="hsb", name="hsb")
                nc.scalar.activation(hsb[:], hps[:], AF.Relu, bias=fc1b[:])
                for cc in range(2):
                    zps = seps.tile([P, 1], F32, tag="se2", name="se_z")
                    nc.tensor.matmul(zps[:], fc2w[:, cc, :], hsb[:],
                                     start=True, stop=True)
                    esb = sing.tile([P, 1], F32, tag=f"esb_{cc}", name=f"esb_{cc}")
                    nc.scalar.activation(esb[:], zps[:], AF.Exp,
                                         bias=fc2bn[:, cc:cc + 1], scale=-1.0)
                    nc.vector.tensor_scalar_add(esb[:], esb[:], 1.0)
                    nc.vector.reciprocal(chw[:, cc:cc + 1], esb[:])

            # ---- main attention loop (software-pipelined) ----
            with tc.tile_pool(name="lg", bufs=2, space="PSUM") as lg, \
                 tc.tile_pool(name="otp", bufs=2, space="PSUM") as otp, \
                 tc.tile_pool(name="bcp", bufs=1, space="PSUM") as bcp, \
                 tc.tile_pool(name="ypjp", bufs=1, space="PSUM") as ypjp, \
                 tc.tile_pool(name="pTp", bufs=2) as pTp, \
                 tc.tile_pool(name="treep", bufs=2) as treep, \
                 tc.tile_pool(name="sumsp", bufs=2) as sumsp, \
                 tc.tile_pool(name="invp", bufs=3) as invp, \
                 tc.tile_pool(name="onp", bufs=2) as onp, \
                 tc.tile_pool(name="bcsbp", bufs=2) as bcsbp:

                # per-block state carried across the deferred pipeline
                st = [dict() for _ in range(NBLK)]

                def emit_logits_group(i, g):
                    # PE: 4 logits MMs for group g of block i
                    s = st[i]
                    th_sl = theta[:, i * nb:(i + 1) * nb]
                    lgt = lg.tile([P, CG, nb], F32, tag="lg", name="lg")
                    s.setdefault("lgt", {})[g] = lgt
                    for j in range(CG):
                        mj = g * CG + j
                        nc.tensor.matmul(lgt[:, j, :],
                                         phi[:, mj * P:(mj + 1) * P], th_sl,
                                         start=True, stop=True)

                def emit_exp_group(i, g):
                    # ACT: exp of group g -> pT slab
                    s = st[i]
                    lgt = s["lgt"].pop(g)
                    nc.scalar.activation(
                        s["pT"][:, g * CG * nb:(g + 1) * CG * nb],
                        lgt[:, :, :].rearrange("p a b -> p (a b)"),
                        AF.Exp, scale=sm_scale)

                def emit_outT_group(i, g):
                    # PE: 4 outT accumulation MMs for group g
                    s = st[i]
                    pT = s["pT"]
                    for j in range(CG):
                        mj = g * CG + j
                        nc.tensor.matmul(
                            s["outT"][:, 0:nb], gT[:, mj * I:(mj + 1) * I],
                            pT[:, mj * nb:(mj + 1) * nb],
                            start=(mj == 0), stop=(mj == M - 1))

                def emit_tree_partial(i, jp, eng=None):
                    # 3-level bf16 pairwise tree over 2 exp groups (SBUF only,
                    # so GPSIMD is legal here — used for partial 0)
                    s = st[i]
                    pT, t1 = s["pT"], s["t1"]
                    e = eng if eng is not None else nc.vector
                    w0 = jp * 2 * CG * nb                 # 2048-col window
                    A = t1[:, jp, 0:1024]
                    Bb = t1[:, jp, 1024:1536]
                    e.tensor_tensor(A, pT[:, w0:w0 + 1024],
                                    pT[:, w0 + 1024:w0 + 2048], ALU.add)
                    e.tensor_tensor(Bb, A[:, 0:512], A[:, 512:1024], ALU.add)
                    e.tensor_tensor(s["s4"][:, jp, :], Bb[:, 0:256],
                                    Bb[:, 256:512], ALU.add)

                def emit_tree_combine(i):
                    s = st[i]
                    s4 = s["s4"].rearrange("p a b -> p (a b)")
                    c2 = s["c2"]
                    nc.vector.tensor_tensor(c2[:, 0:512], s4[:, 0:512],
                                            s4[:, 512:1024], ALU.add)
                    nc.vector.tensor_tensor(s["sums"][:], c2[:, 0:256],
                                            c2[:, 256:512], ALU.add)

                def emit_cs(i):
                    # PE: partition-reduce of sums via ones matmul
                    s = st[i]
                    nc.tensor.matmul(s["cs"], ones_cb[:], s["sums"][:],
                                     start=True, stop=True)

                def emit_recip(i):
                    s = st[i]
                    nc.vector.reciprocal(s["inv"][:], s["cs"])

                def emit_bc_mm(i):
                    # PE: broadcast 1/denom to 128 partitions (fp32 MM)
                    s = st[i]
                    nc.tensor.matmul(s["bc"][:], ones_row[:], s["inv"][:],
                                     start=True, stop=True)

                def emit_bcast(i):
                    # ACT: bc PSUM->SBUF copy
                    s = st[i]
                    nc.scalar.copy(s["bc_sb"][:], s["bc"][:])

                def emit_on(i):
                    # DVE: on = outT * (1/denom broadcast)
                    s = st[i]
                    nc.vector.tensor_tensor(s["on"][:], s["outT"][:, 0:nb],
                                            s["bc_sb"][:], ALU.mult)

                def emit_ypj(i):
                    s = st[i]
                    for cc in range(2):
                        nc.tensor.matmul(s["ypj"][:, cc, :], wo[:, cc, :],
                                         s["on"][:], start=True, stop=True)

                def emit_drain(i):
                    # DVE: whole-bank ypj drain to bf16 ysb
                    s = st[i]
                    nc.vector.tensor_scalar(
                        ysb[:, :, i * nb:(i + 1) * nb], s["ypj"][:, :, :],
                        1.0, None, ALU.mult)
                    s.pop("ypj")

                def emit_stats(i):
                    for cc in range(2):
                        nc.vector.bn_stats(
                            out=bnst[cc][:, i, :],
                            in_=ysb[:, cc, i * nb:(i + 1) * nb])

                def alloc_block(i):
                    s = st[i]
                    s["pT"] = pTp.tile([P, M * nb], BF16, tag="pT", name="pT")
                    s["t1"] = treep.tile([P, NPART, 1536], BF16, tag="t1",
                                         name="t1")
                    s["s4"] = treep.tile([P, NPART, nb], BF16, tag="s4",
                                         name="s4")
                    s["c2"] = treep.tile([P, 512], BF16, tag="c2", name="c2")
                    s["sums"] = sumsp.tile([P, nb], BF16, tag="sums",
                                           name="sums")
                    s["inv"] = invp.tile([1, nb], F32, tag="inv", name="inv")
                    ot = otp.tile([P, 512], F32, tag="outT", name="outT")
                    s["outT"] = ot
                    s["cs"] = ot[0:1, 256:512][:, 0:nb]
                    s["bc"] = bcp.tile([P, nb], F32, tag="bc", name="bc")
                    s["bc_sb"] = bcsbp.tile([P, nb], F32, tag="bc_sb",
                                            name="bc_sb")
                    s["on"] = onp.tile([P, nb], BF16, tag="on", name="on")
                    s["ypj"] = ypjp.tile([P, 2, nb], F32, tag="ypj", name="ypj")

                # software pipeline, stages deferred a full block each:
                #  iter i:   L(i) paced by exp(i) [ACT]
                #  iter i+1: O(i), tree(i), cs(i), recip(i)   (exp(i) done —
                #            nothing here waits on ACT)
                #  iter i+2: bcast(i), on(i), ypj(i), drain(i)
                #  iter i+3: stats(i)
                for i in range(NBLK + 4):
                    L = i if i < NBLK else None
                    O = i - 1 if 0 <= i - 1 < NBLK else None
                    Y = i - 2 if 0 <= i - 2 < NBLK else None
                    S = i - 3 if 0 <= i - 3 < NBLK else None
                    if L is not None:
                        alloc_block(L)
                    # bc chain for block i-2 first (PE MM + ACT copy), then
                    # GP partial p0(i-1)
                    if Y is not None:
                        emit_bc_mm(Y)
                        emit_bcast(Y)
                    if O is not None:
                        emit_tree_partial(O, 0, eng=nc.gpsimd)
                    # DVE: ungated tree work first, chain ops later
                    if O is not None:
                        emit_tree_partial(O, 1)
                    if Y is not None:
                        emit_on(Y)
                    if L is not None:
                        emit_logits_group(L, 0)
                        emit_exp_group(L, 0)
                        emit_logits_group(L, 1)
                        emit_exp_group(L, 1)
                    if O is not None:
                        emit_outT_group(O, 0)
                        emit_outT_group(O, 1)
                        emit_tree_partial(O, 2)
                    if L is not None:
                        emit_logits_group(L, 2)
                        emit_exp_group(L, 2)
                    if O is not None:
                        emit_outT_group(O, 2)
                    if L is not None:
                        emit_logits_group(L, 3)
                        emit_exp_group(L, 3)
                    if O is not None:
                        emit_outT_group(O, 3)
                        emit_tree_partial(O, 3)
                    if Y is not None:
                        emit_ypj(Y)
                    if L is not None:
                        emit_logits_group(L, 4)
                        emit_exp_group(L, 4)
                    if O is not None:
                        emit_outT_group(O, 4)
                        emit_tree_combine(O)
                    if L is not None:
                        emit_logits_group(L, 5)
                        emit_exp_group(L, 5)
                    if O is not None:
                        emit_outT_group(O, 5)
                    if Y is not None:
                        emit_drain(Y)
                    if S is not None:
                        emit_stats(S)
                    if L is not None:
                        emit_logits_group(L, 6)
                        emit_exp_group(L, 6)
                    if O is not None:
                        emit_outT_group(O, 6)
                    if L is not None:
                        emit_logits_group(L, 7)
                        emit_exp_group(L, 7)
                    if O is not None:
                        emit_outT_group(O, 7)
                        emit_cs(O)
                        emit_recip(O)

            # ---- epilogue: sync-BN, affine, residual ----
            with tc.tile_pool(name="epi", bufs=2) as epi:
                stats = sing.tile([P, 4], F32, tag="stats", name="stats")
                for cc in range(2):
                    mv = epi.tile([P, 2], F32, tag="mv", name="mv")
                    nc.vector.bn_aggr(out=mv[:], in_=bnst[cc][:, :, :])
                    # sum = mean*nn ; sumsq = (var + mean^2)*nn
                    nc.vector.tensor_scalar_mul(stats[:, cc:cc + 1],
                                                mv[:, 0:1], float(nn))
                    m2 = epi.tile([P, 1], F32, tag="m2", name="m2")
                    nc.vector.tensor_tensor(m2[:], mv[:, 0:1], mv[:, 0:1],
                                            ALU.mult)
                    nc.vector.tensor_tensor(m2[:], mv[:, 1:2], m2[:], ALU.add)
                    nc.vector.tensor_scalar_mul(stats[:, 2 + cc:3 + cc],
                                                m2[:], float(nn))
                nc.sync.dma_start(out=bn_in[:], in_=stats[:])
                nc.gpsimd.collective_compute(
                    "AllReduce", ALU.add,
                    replica_groups=[list(range(n_cores))],
                    ins=[bn_in[:]], outs=[bn_out[:]])
                stats_g = sing.tile([P, 4], F32, tag="stats_g", name="stats_g")
                nc.gpsimd.dma_start(out=stats_g[:], in_=bn_out[:])

                inv_np = 1.0 / float(total_pixels)
                qn = nn // 2
                for cc in range(2):
                    mean = epi.tile([P, 1], F32, tag="mean", name="mean")
                    ex2 = epi.tile([P, 1], F32, tag="ex2", name="ex2")
                    nc.vector.tensor_scalar_mul(mean[:], stats_g[:, cc:cc + 1],
                                                inv_np)
                    nc.vector.tensor_scalar_mul(ex2[:], stats_g[:, 2 + cc:3 + cc],
                                                inv_np)
                    var = epi.tile([P, 1], F32, tag="var", name="var")
                    nc.vector.tensor_tensor(var[:], mean[:], mean[:], ALU.mult)
                    nc.vector.tensor_tensor(var[:], ex2[:], var[:], ALU.subtract)
                    nc.vector.tensor_scalar_add(var[:], var[:], 1e-5)
                    lnv = epi.tile([P, 1], F32, tag="lnv", name="lnv")
                    nc.scalar.activation(lnv[:], var[:], AF.Ln)
                    istd = epi.tile([P, 1], F32, tag="istd", name="istd")
                    nc.scalar.activation(istd[:], lnv[:], AF.Exp, scale=-0.5)
                    g1 = epi.tile([P, 1], F32, tag="g1", name="g1")
                    nc.vector.tensor_tensor(g1[:], istd[:], gam[:, cc:cc + 1],
                                            ALU.mult)
                    A = epi.tile([P, 1], F32, tag="A", name="A")
                    nc.vector.tensor_tensor(A[:], g1[:], chw[:, cc:cc + 1],
                                            ALU.mult)
                    Bt = epi.tile([P, 1], F32, tag="Bt", name="Bt")
                    nc.vector.tensor_tensor(Bt[:], mean[:], g1[:], ALU.mult)
                    nc.vector.tensor_tensor(Bt[:], bet[:, cc:cc + 1], Bt[:],
                                            ALU.subtract)
                    nc.vector.tensor_tensor(Bt[:], Bt[:], chw[:, cc:cc + 1],
                                            ALU.mult)

                    # apply: tf = ysb*A + B (DVE 4x bf16), osb = tf + xbf
                    # (DVE 2x bf16), chunked bf16 DMA out
                    tf = epi.tile([P, nn], BF16, tag="tf", name="tf")
                    nc.vector.tensor_scalar(tf[:], ysb[:, cc, :], A[:], Bt[:],
                                            ALU.mult, ALU.add)
                    for h in range(2):
                        sl = slice(h * qn, (h + 1) * qn)
                        osb = epi.tile([P, qn], BF16, tag="osb", name="osb")
                        nc.vector.tensor_tensor(osb[:], tf[:, sl],
                                                xbf[cc][:, sl], ALU.add)
                        nc.sync.dma_start(out=out_d[cc * P:(cc + 1) * P, sl],
                                          in_=osb[:])

    nc.compile()
    return nc


_NC_CACHE = {}


def _get_nc():
    if "nc" not in _NC_CACHE:
        _NC_CACHE["nc"] = _build()
    return _NC_CACHE["nc"]


def _prep_inputs(x_b, theta_w, phi_w, g_w, out_w, bn_gamma, bn_beta,
                 fc1_w, fc1_b, fc2_w, fc2_b):
    bf = ml_dtypes.bfloat16
    return {
        "x": np.ascontiguousarray(x_b, dtype=np.float32),
        "wq_t": np.ascontiguousarray(np.asarray(theta_w, np.float32).T).astype(bf),
        "wk_t": np.ascontiguousarray(np.asarray(phi_w, np.float32).T).astype(bf),
        "wg_t": np.ascontiguousarray(np.asarray(g_w, np.float32).T).astype(bf),
        "wo_t": np.ascontiguousarray(np.asarray(out_w, np.float32).T).astype(bf),
        "fc1_wt": np.ascontiguousarray(
            (np.asarray(fc1_w, np.float32) / N).T).astype(np.float32),
        "fc1_b": np.ascontiguousarray(fc1_b, dtype=np.float32),
        "fc2_wt": np.ascontiguousarray(
            np.asarray(fc2_w, np.float32).T).astype(np.float32),
        "fc2_bn": np.ascontiguousarray(-np.asarray(fc2_b, np.float32)),
        "bn_gamma": np.ascontiguousarray(bn_gamma, dtype=np.float32),
        "bn_beta": np.ascontiguousarray(bn_beta, dtype=np.float32),
    }


def _run(inputs, trace=False):
    nc = _get_nc()
    x = np.asarray(inputs["x"], dtype=np.float32)
    xs = x.reshape(B, C, N)
    in_maps = [
        _prep_inputs(xs[i], inputs["theta_w"], inputs["phi_w"], inputs["g_w"],
                     inputs["out_w"], inputs["bn_gamma"], inputs["bn_beta"],
                     inputs["fc1_w"], inputs["fc1_b"], inputs["fc2_w"],
                     inputs["fc2_b"])
        for i in range(B)
    ]
    res = run_bass_kernel_spmd(nc, in_maps, list(range(B)), trace=trace)
    out = np.stack([np.asarray(res.results[i]["out"]).astype(np.float32)
                    for i in range(B)])
    return out.reshape(B, C, H, W), res


def kernel(**inputs) -> np.ndarray:
    out, _ = _run(inputs, trace=False)
    return out
